# revision 3
# baseline (speedup 1.0000x reference)
"""Trainium2 Bass kernel for 16-head self-attention (B=1, T=2048, d=1024).

Sharding: 2 heads per NeuronCore (tensor-parallel over QKV columns / proj
rows) across 8 cores; each core emits a partial [T, d] projection output
(bf16), summed on the host.

v2 design (vs baseline):
  - q/k projections chunk-outer (stationary weight reused across the 4
    q-groups; 8 psum accumulators live at once).
  - v tiles are 32 separate [128, 65] sbuf tiles ([v|1] per head); the
    later ones are computed interleaved into group 0's attention loop.
  - softmax normalization folded into the y psum -> sbuf staging copies:
    sums row -> reciprocal (DVE) -> replicated across partitions via a
    K=1 matmul with a ones vector -> one tensor_mul per head.
  - projection is a single K=128 matmul per tile (yT holds both heads,
    already normalized); outputs staged as bf16 and DMAd out.
"""
import os
import sys

sys.path.insert(0, "/opt/trn_rl_repo")

import numpy as np
import ml_dtypes

import concourse.bass as bass
import concourse.bacc as bacc
import concourse.tile as tile
from concourse import mybir
from concourse import bass_utils

BF16 = mybir.dt.bfloat16
F32 = mybir.dt.float32

B, T, D = 1, 2048, 1024
H = 16
HD = D // H          # 64
NCORES = 8
HPD = H // NCORES    # 2 heads per device
DL = HPD * HD        # 128 local head dims per device
QG = 4               # q groups of 512
QGS = T // QG        # 512
KT = T // 128        # 16 k tiles
NG = D // 512        # 2 output column groups

last_results = None  # BassKernelResults of the most recent run (for test.py)

DEBUG_DUMPS = os.environ.get("KERNEL_DEBUG_DUMPS", "0") == "1"


def build_nc():
    nc = bacc.Bacc("TRN2", target_bir_lowering=False, debug=False,
                   num_devices=NCORES)
    xT = nc.dram_tensor("xT", [D, T], BF16, kind="ExternalInput").ap()
    wq = nc.dram_tensor("wq", [D, DL], BF16, kind="ExternalInput").ap()
    wk = nc.dram_tensor("wk", [D, DL], BF16, kind="ExternalInput").ap()
    wv = nc.dram_tensor("wv", [D, DL], BF16, kind="ExternalInput").ap()
    wp = nc.dram_tensor("wp", [DL, D], BF16, kind="ExternalInput").ap()
    out = nc.dram_tensor("out", [T, D], BF16, kind="ExternalOutput").ap()
    dbg = {}
    if DEBUG_DUMPS:
        for name, shape, dt in [
            ("dbg_qT", [128, T], BF16), ("dbg_kT", [128, T], BF16),
            ("dbg_yT", [128, T], BF16),
            ("dbg_rr", [1, QG, 2 * QGS], BF16),
        ]:
            dbg[name] = nc.dram_tensor(name, shape, dt,
                                       kind="ExternalOutput").ap()

    with tile.TileContext(nc) as tc:
        with (
            tc.tile_pool(name="const", bufs=1) as cpool,
            tc.tile_pool(name="vpool", bufs=1) as vpool,
            tc.tile_pool(name="epool", bufs=3) as epool,
            tc.tile_pool(name="spool", bufs=4) as spool,
            tc.tile_pool(name="rpool", bufs=2) as rpool,
            tc.tile_pool(name="wpool", bufs=2) as wpool,
            tc.tile_pool(name="opool", bufs=4) as opool,
            tc.tile_pool(name="sc", bufs=2, space="PSUM") as scp,    # 2x2 banks
            tc.tile_pool(name="yp", bufs=2, space="PSUM") as yp,     # 2x1 bank
            tc.tile_pool(name="pj", bufs=2, space="PSUM") as pjp,    # 2x1 bank
        ):
            # ---- persistent SBUF tensors ----
            xT_sb = cpool.tile([128, 8, T], BF16)       # x^T, d-major chunks
            wq_sb = cpool.tile([128, 8, DL], BF16)
            wk_sb = cpool.tile([128, 8, DL], BF16)
            wv_sb = cpool.tile([128, 8, DL], BF16)
            wp_sb = cpool.tile([128, D], BF16)          # proj rows (both heads)
            qT_sb = cpool.tile([128, T], BF16)
            kT_sb = cpool.tile([128, T], BF16)
            yT_sb = cpool.tile([128, T], BF16)          # normalized y^T
            ones_sb = cpool.tile([1, 128], BF16)
            # v tiles: [t, v|1] per k-tile per head, ones in col 64
            v0_t = [vpool.tile([128, 65], BF16, name=f"v0_{tt}")
                    for tt in range(KT)]
            v1_t = [vpool.tile([128, 65], BF16, name=f"v1_{tt}")
                    for tt in range(KT)]

            # ---- input DMAs (split along d so matmuls can start early) ----
            xTr = xT.rearrange("(n p) t -> p n t", p=128)
            wqr = wq.rearrange("(n p) m -> p n m", p=128)
            wkr = wk.rearrange("(n p) m -> p n m", p=128)
            wvr = wv.rearrange("(n p) m -> p n m", p=128)
            for kk in range(8):
                eng = nc.sync if kk % 2 == 0 else nc.gpsimd
                eng.dma_start(xT_sb[:, kk, :], xTr[:, kk, :])
                eng.dma_start(wq_sb[:, kk, :], wqr[:, kk, :])
                eng.dma_start(wk_sb[:, kk, :], wkr[:, kk, :])
                eng.dma_start(wv_sb[:, kk, :], wvr[:, kk, :])
            nc.sync.dma_start(wp_sb[:], wp[:, :])
            nc.vector.memset(ones_sb[:], 1.0)
            for tt in range(KT):
                nc.gpsimd.memset(v0_t[tt][:], 1.0)
                nc.gpsimd.memset(v1_t[tt][:], 1.0)

            # ---- phase 1a: q/k, chunk-outer (stationary w reused 4x) ----
            qA = scp.tile([128, 2 * QGS], F32, tag="sc")   # q g0|g1
            qB = scp.tile([128, 2 * QGS], F32, tag="sc")   # q g2|g3
            kacc = [yp.tile([128, QGS], F32, tag="y", name=f"k{g}")
                    for g in range(2)]
            kacc += [pjp.tile([128, QGS], F32, tag="pj", name=f"k{g + 2}")
                     for g in range(2)]
            for kk in range(8):
                st, sp = (kk == 0), (kk == 7)
                for g in range(2):
                    nc.tensor.matmul(qA[:, g * QGS:(g + 1) * QGS],
                                     wq_sb[:, kk, :],
                                     xT_sb[:, kk, g * QGS:(g + 1) * QGS],
                                     start=st, stop=sp)
                for g in range(2):
                    nc.tensor.matmul(qB[:, g * QGS:(g + 1) * QGS],
                                     wq_sb[:, kk, :],
                                     xT_sb[:, kk, (g + 2) * QGS:(g + 3) * QGS],
                                     start=st, stop=sp)
                for g in range(4):
                    nc.tensor.matmul(kacc[g][:], wk_sb[:, kk, :],
                                     xT_sb[:, kk, g * QGS:(g + 1) * QGS],
                                     start=st, stop=sp)
            nc.vector.tensor_copy(qT_sb[:, 0:2 * QGS], qA[:])
            nc.vector.tensor_copy(qT_sb[:, 2 * QGS:4 * QGS], qB[:])
            for g in range(4):
                nc.vector.tensor_copy(kT_sb[:, g * QGS:(g + 1) * QGS],
                                      kacc[g][:])

            # ---- v tiles (head-dim columns 0:64; col 64 stays 1.0) ----
            def emit_v_tile(tt):
                vp = pjp.tile([128, DL], F32, tag="pj", name=f"vp{tt}")
                for kk in range(8):
                    nc.tensor.matmul(vp[:],
                                     xT_sb[:, kk, tt * 128:(tt + 1) * 128],
                                     wv_sb[:, kk, :],
                                     start=(kk == 0), stop=(kk == 7))
                nc.vector.tensor_copy(v0_t[tt][:, 0:HD], vp[:, 0:HD])
                nc.vector.tensor_copy(v1_t[tt][:, 0:HD], vp[:, HD:2 * HD])

            NV_PRE = 4               # v tiles computed before group 0 starts
            for tt in range(NV_PRE):
                emit_v_tile(tt)

            # ---- phase 2: attention + projection per q-group ----
            for g in range(QG):
                qsl = slice(g * QGS, (g + 1) * QGS)
                y0 = yp.tile([65, QGS], F32, tag="y")
                y1 = yp.tile([65, QGS], F32, tag="y")
                for kk in range(KT):
                    if g == 0 and NV_PRE + kk < KT:
                        emit_v_tile(NV_PRE + kk)
                    ksl = slice(kk * 128, (kk + 1) * 128)
                    sc = scp.tile([128, 2 * QGS], F32, tag="sc")
                    nc.tensor.matmul(sc[:, 0:QGS], kT_sb[0:64, ksl],
                                     qT_sb[0:64, qsl], start=True,
                                     stop=True, tile_position=(0, 0))
                    nc.tensor.matmul(sc[:, QGS:2 * QGS], kT_sb[64:128, ksl],
                                     qT_sb[64:128, qsl], start=True,
                                     stop=True, tile_position=(64, 0))
                    e = epool.tile([128, 2 * QGS], BF16, tag="e")
                    nc.scalar.activation(e[:], sc[:],
                                         mybir.ActivationFunctionType.Exp)
                    nc.tensor.matmul(y0[:], v0_t[kk][:], e[:, 0:QGS],
                                     start=(kk == 0), stop=(kk == KT - 1))
                    nc.tensor.matmul(y1[:], v1_t[kk][:], e[:, QGS:2 * QGS],
                                     start=(kk == 0), stop=(kk == KT - 1))

                # epilogue: stage y to sbuf, normalize into yT
                st0 = spool.tile([65, QGS], F32, tag="st")
                st1 = spool.tile([65, QGS], F32, tag="st")
                nc.vector.tensor_copy(st0[:], y0[:])
                nc.vector.tensor_copy(st1[:], y1[:])
                rr = rpool.tile([1, 2 * QGS], BF16, tag="rr")
                with nc.allow_low_precision(
                        reason="softmax 1/sum in bf16; 0.4% rel err ok"):
                    nc.vector.reciprocal(rr[0:1, 0:QGS], st0[64:65, :])
                    nc.vector.reciprocal(rr[0:1, QGS:2 * QGS], st1[64:65, :])
                if DEBUG_DUMPS:
                    nc.sync.dma_start(dbg["dbg_rr"][0, g, :], rr[0:1, :])
                rep0 = pjp.tile([128, QGS], F32, tag="pj")
                rep1 = pjp.tile([128, QGS], F32, tag="pj")
                nc.tensor.matmul(rep0[:], ones_sb[:], rr[0:1, 0:QGS],
                                 start=True, stop=True)
                nc.tensor.matmul(rep1[:], ones_sb[:], rr[0:1, QGS:2 * QGS],
                                 start=True, stop=True)
                nc.vector.tensor_mul(yT_sb[0:64, qsl], st0[0:64, :],
                                     rep0[0:64, :])
                ytmp = wpool.tile([64, QGS], BF16, tag="ytmp")
                nc.vector.tensor_mul(ytmp[:], st1[0:64, :], rep1[0:64, :])
                nc.gpsimd.dma_start(yT_sb[64:128, qsl], ytmp[:])

                # projection for this q-group (overlaps next group's k-loop)
                for qt in range(g * 4, (g + 1) * 4):
                    tsl = slice(qt * 128, (qt + 1) * 128)
                    for ngi in range(NG):
                        nsl = slice(ngi * 512, (ngi + 1) * 512)
                        pj = pjp.tile([128, 512], F32, tag="pj")
                        nc.tensor.matmul(pj[:], yT_sb[:, tsl], wp_sb[:, nsl],
                                         start=True, stop=True)
                        ot = opool.tile([128, 512], BF16, tag="o")
                        nc.vector.tensor_copy(ot[:], pj[:])
                        (nc.sync if (qt + ngi) % 2 else nc.gpsimd).dma_start(
                            out[tsl, nsl], ot[:])

            if DEBUG_DUMPS:
                nc.sync.dma_start(dbg["dbg_qT"][:], qT_sb[:])
                nc.sync.dma_start(dbg["dbg_kT"][:], kT_sb[:])
                nc.sync.dma_start(dbg["dbg_yT"][:], yT_sb[:])

    nc.compile()
    return nc


_nc_cache = None


def kernel(x: np.ndarray, W_qkv: np.ndarray, W_proj: np.ndarray) -> np.ndarray:
    global _nc_cache, last_results
    assert x.shape == (B, T, D)
    x2d = np.ascontiguousarray(x.reshape(T, D))
    xT = np.ascontiguousarray(x2d.T).astype(ml_dtypes.bfloat16)
    scale = 1.0 / np.sqrt(np.float32(HD))

    in_maps = []
    for dev in range(NCORES):
        wq_ = (W_qkv[:, dev * DL:(dev + 1) * DL] * scale).astype(
            ml_dtypes.bfloat16)
        wk_ = W_qkv[:, D + dev * DL: D + (dev + 1) * DL].astype(
            ml_dtypes.bfloat16)
        wv_ = W_qkv[:, 2 * D + dev * DL: 2 * D + (dev + 1) * DL].astype(
            ml_dtypes.bfloat16)
        wp_ = W_proj[dev * DL:(dev + 1) * DL, :].astype(ml_dtypes.bfloat16)
        in_maps.append({
            "xT": xT,
            "wq": np.ascontiguousarray(wq_),
            "wk": np.ascontiguousarray(wk_),
            "wv": np.ascontiguousarray(wv_),
            "wp": np.ascontiguousarray(wp_),
        })

    if _nc_cache is None:
        _nc_cache = build_nc()
    res = bass_utils.run_bass_kernel_spmd(
        _nc_cache, in_maps, core_ids=list(range(NCORES)),
        tmpdir=os.environ.get("BASS_KERNEL_TMPDIR"))
    last_results = res
    total = np.zeros((T, D), dtype=np.float32)
    for dev in range(NCORES):
        total += res.results[dev]["out"].astype(np.float32)
    return total.reshape(B, T, D)


if __name__ == "__main__":
    rng = np.random.default_rng(0)
    x = rng.standard_normal((B, T, D)).astype(np.float32)
    wqkv = (rng.standard_normal((D, 3 * D)) * 0.02).astype(np.float32)
    wproj = (rng.standard_normal((D, D)) * 0.02).astype(np.float32)
    y = kernel(x, wqkv, wproj)
    print("kernel output", y.shape, y.dtype, float(np.abs(y).mean()))
